# revision 38
# baseline (speedup 1.0000x reference)
"""GQA causal-attention prefill kernel for Trainium2, tensor-parallel over 8 NeuronCores.

v8: all-bf16 datapath, single fused PE stream with o-projection filler.

Sharding: head-parallel. Core c gets q heads [4c, 4c+4), kv head c, and the
matching wo slice; each core computes a full-shape partial output
o_part = attn(heads of c) @ wo_c (bf16) and the host sums the 8 partials.

Per-core schedule, per (batch, 512-token chunk):
  sweep:   qkv projections, 6 PSUM groups over a 32-step k sweep
  release: each group leaves PSUM fast (ACT direct copy + DVE half-swap
           copies to bf16 staging) so all 8 banks are free ~4us after the
           sweep; rope math (3 DVE ops/group: t=swap*sinS, c=direct*cos,
           dst=t+c) then runs off-bank.  sinS has its first partition half
           negated so rope needs no subtract.
  attn:    transposed scores -> exp on ACT (bf16) -> av / denominator
           matmuls, software-pipelined one block deep; causal diagonal
           blocks column-trimmed and masked by a 0/1 wedge mul on DVE.
           The PREVIOUS chunk's o-projection tiles are interleaved one per
           attention block as PE filler, absorbing every cross-engine
           latency (exp, mask, bank handover).
  denom:   lbc[m,t] += ones[128,128].T @ p broadcasts the column sum to
           all partitions; 1/l = exp(-ln(l)) once per chunk on ACT (DVE
           reciprocal is ~8 cycles/elem; per-head Ln would thrash the
           activation table, 1.3us per reload).  The outT=av*rl muls are
           emitted in the NEXT chunk's s1 so DVE never waits on ACT.
"""

import os
import sys

sys.path.insert(0, "/opt/trn_rl_repo")

import numpy as np

B = 2
T = 2048
TOK = B * T
D = 4096
NQ = 32
NKV = 8
H = 128
HH = H // 2
THETA = 10000.0
NCORES = 8
NHC = NQ // NCORES          # q heads per core (4)
KPC = D // H                # contraction chunks of 128 over D (32)
TCH = 512                   # token chunk
NTCH = T // TCH             # 4 token chunks per batch
NSUB = TCH // H             # 4 128-wide subtiles per chunk
C_SM = 1.0 / np.sqrt(H)     # softmax scale


def _build_bass():
    import concourse.bacc as bacc
    import concourse.mybir as mybir
    import concourse.tile as tile
    from concourse.masks import make_identity
    from contextlib import ExitStack

    f32 = mybir.dt.float32
    bf16 = mybir.dt.bfloat16
    Exp = mybir.ActivationFunctionType.Exp
    Ln = mybir.ActivationFunctionType.Ln
    Copy = mybir.ActivationFunctionType.Copy

    nc = bacc.Bacc("TRN2", target_bir_lowering=False, debug=False,
                   num_devices=NCORES)

    xT = nc.declare_dram_parameter("xT", [D, TOK], bf16, isOutput=False)
    wq = nc.declare_dram_parameter("wq", [NHC, D, H], bf16, isOutput=False)
    wk = nc.declare_dram_parameter("wk", [D, H], bf16, isOutput=False)
    wv = nc.declare_dram_parameter("wv", [D, H], bf16, isOutput=False)
    wo = nc.declare_dram_parameter("wo", [NHC, H, D], bf16, isOutput=False)
    cosT = nc.declare_dram_parameter("cosT", [H, TOK], f32, isOutput=False)
    sinST = nc.declare_dram_parameter("sinST", [H, TOK], f32, isOutput=False)
    o_part = nc.declare_dram_parameter("o_part", [TOK, D], bf16,
                                       isOutput=True)

    with tile.TileContext(nc) as tc:
        with ExitStack() as top:
            consts = top.enter_context(tc.tile_pool(name="consts", bufs=1))
            identity = consts.tile([H, H], bf16)
            make_identity(nc, identity)
            ones_sq = consts.tile([H, H], bf16, tag="ones")
            nc.vector.memset(ones_sq, 1.0)
            # 0/1 causal wedge masks: mask[j][s, t] = 1 iff (t - s - 128j) >= 0
            masks = []
            for j in range(NSUB):
                m = consts.tile([H, TCH], bf16, tag=f"mask{j}",
                                name=f"mask{j}")
                nc.vector.memset(m, 1.0)
                nc.gpsimd.affine_select(
                    out=m, in_=m,
                    compare_op=mybir.AluOpType.is_ge,
                    fill=0.0,
                    base=-H * j,
                    pattern=[[1, TCH]],
                    channel_multiplier=-1,
                )
                masks.append(m)

            # ---- persistent weights / tables ----
            wpool = top.enter_context(tc.tile_pool(name="wpool", bufs=1))
            wq_src = wq.rearrange("h (c p) m -> p h c m", p=H)
            wqs = [wpool.tile([H, KPC, H], bf16, tag=f"wq{i}",
                              name=f"wq{i}") for i in range(NHC)]
            wk_sb = wpool.tile([H, KPC, H], bf16, tag="wk")
            wk_src = wk.rearrange("(c p) m -> p c m", p=H)
            wv_sb = wpool.tile([H, KPC, H], bf16, tag="wv")
            wv_src = wv.rearrange("(c p) m -> p c m", p=H)

            def emit_w_dmas(c8):
                sl = slice(c8 * 8, (c8 + 1) * 8)
                for i in range(NHC):
                    nc.sync.dma_start(out=wqs[i][:, sl, :],
                                      in_=wq_src[:, i, sl, :])
                nc.sync.dma_start(out=wk_sb[:, sl, :], in_=wk_src[:, sl, :])
                nc.sync.dma_start(out=wv_sb[:, sl, :], in_=wv_src[:, sl, :])

            emit_w_dmas(0)
            cos_sb = wpool.tile([H, T], f32, tag="cos")
            sin_sb = wpool.tile([H, T], f32, tag="sin")

            def emit_table_dmas(b):
                for c4 in range(4):
                    sl = slice(c4 * TCH, (c4 + 1) * TCH)
                    gsl = slice(b * T + c4 * TCH, b * T + (c4 + 1) * TCH)
                    nc.sync.dma_start(out=cos_sb[:, sl], in_=cosT[:, gsl])
                    nc.sync.dma_start(out=sin_sb[:, sl], in_=sinST[:, gsl])

            wo_sb = wpool.tile([H, NHC, D], bf16, tag="wo")
            wo_src = wo.rearrange("h p d -> p h d")

            def emit_wo_dmas():
                for dc8 in range(8):
                    sl = slice(dc8 * TCH, (dc8 + 1) * TCH)
                    nc.sync.dma_start(out=wo_sb[:, :, sl],
                                      in_=wo_src[:, :, sl])

            # ---- persistent activations ----
            act = top.enter_context(tc.tile_pool(name="act", bufs=1))
            qTs = [act.tile([H, NHC, TCH], bf16, tag=f"qT{i}",
                            name=f"qT{i}") for i in range(NTCH)]
            kTs = [act.tile([H, TCH], bf16, tag=f"kT{i}",
                            name=f"kT{i}") for i in range(NTCH)]
            vs = [act.tile([H, NSUB, H], bf16, tag=f"v{i}",
                           name=f"v{i}") for i in range(NTCH)]

            xpool = top.enter_context(tc.tile_pool(name="xpool", bufs=18))
            rtmp = top.enter_context(tc.tile_pool(name="rtmp", bufs=2))
            vstg = top.enter_context(tc.tile_pool(name="vstg", bufs=2))
            ppool = top.enter_context(tc.tile_pool(name="ppool", bufs=4))
            pdiag = top.enter_context(tc.tile_pool(name="pdiag", bufs=2))
            avlpool = top.enter_context(tc.tile_pool(name="avlpool", bufs=1))
            otpool = top.enter_context(tc.tile_pool(name="otpool", bufs=2))
            opool = top.enter_context(tc.tile_pool(name="opool", bufs=5))

            def rope_math(psum, dst, cs, sn):
                # reads PSUM directly: the o-projection window gives the
                # bank ~20us before attention claims it, so no staging.
                tmp = rtmp.tile([H, TCH], f32, tag="rt")
                tmp2 = rtmp.tile([H, TCH], f32, tag="rt2")
                nc.vector.tensor_mul(tmp[0:HH, :], psum[HH:H, :], sn[0:HH, :])
                nc.vector.tensor_mul(tmp[HH:H, :], psum[0:HH, :], sn[HH:H, :])
                nc.vector.tensor_mul(tmp2, psum, cs)
                nc.vector.tensor_add(dst, tmp, tmp2)

            def oproj_tile_emitters(pend):
                pb, pc, outT = pend
                ems = []
                for u in range(NSUB):
                    trow = pb * T + pc * TCH + u * H
                    for dc in range(D // TCH):
                        def em(pool, u=u, dc=dc, trow=trow, outT=outT):
                            ops = pool.tile([H, TCH], f32, tag="o")
                            for h in range(NHC):
                                nc.tensor.matmul(
                                    ops,
                                    outT[:, h, u * H:(u + 1) * H],
                                    wo_sb[:, h, dc * TCH:(dc + 1) * TCH],
                                    start=(h == 0), stop=(h == NHC - 1),
                                    skip_group_check=True)
                            o_sb = opool.tile([H, TCH], bf16, tag="osb")
                            # every third eviction on DVE so the ACT queue
                            # drains in time for the next chunk's first exps
                            if (u * 8 + dc) % 3 == 2:
                                nc.vector.tensor_copy(o_sb, ops)
                            else:
                                nc.scalar.activation(o_sb, ops, Copy)
                            nc.sync.dma_start(
                                out=o_part[trow:trow + H,
                                           dc * TCH:(dc + 1) * TCH],
                                in_=o_sb)
                        ems.append(em)
                return ems

            def normalize(av_all, l_all, outT_sb, heads):
                # 1/l = exp(-ln(l)); batched over `heads` so the Exp<->Ln
                # activation-table reloads happen once per call.
                h0, h1 = heads
                nh = h1 - h0
                lg = avlpool.tile([H, NHC, TCH], f32, tag="lg")
                nc.scalar.activation(lg[:, h0:h1, :], l_all[:, h0:h1, :], Ln)
                rl = avlpool.tile([H, NHC, TCH], f32, tag="rl")
                nc.scalar.activation(rl[:, h0:h1, :], lg[:, h0:h1, :], Exp,
                                     scale=-1.0)
                return [(outT_sb, av_all, rl, h) for h in range(h0, h1)]

            def emit_norm_muls(muls):
                for outT_sb, av_all, rl, h in muls:
                    nc.vector.tensor_mul(outT_sb[:, h, :],
                                         av_all[:, h, :], rl[:, h, :])

            pending = None
            pending_muls = []
            for b in range(B):
                for c in range(NTCH):
                    t0 = c * TCH
                    cs = cos_sb[:, t0:t0 + TCH]
                    sn = sin_sb[:, t0:t0 + TCH]
                    first = (b == 0 and c == 0)
                    with ExitStack() as s1:
                        pj = s1.enter_context(
                            tc.tile_pool(name="pj", bufs=1, space="PSUM"))
                        po = s1.enter_context(
                            tc.tile_pool(name="po", bufs=2, space="PSUM"))
                        g = [pj.tile([H, TCH], f32, tag=f"g{i}",
                                     name=f"g{i}") for i in range(6)]
                        for k in range(KPC):
                            x_t = xpool.tile([H, TCH], bf16, tag="x")
                            nc.sync.dma_start(
                                out=x_t,
                                in_=xT[k * H:(k + 1) * H,
                                       b * T + t0:b * T + t0 + TCH])
                            for i in range(6):
                                lhs = (wqs[i][:, k, :] if i < 4 else
                                       (wk_sb if i == 4 else wv_sb)[:, k, :])
                                nc.tensor.matmul(
                                    g[i], lhs, x_t,
                                    start=(k == 0), stop=(k == KPC - 1),
                                    skip_group_check=True)
                            if first:
                                if k in (6, 14, 22):
                                    emit_w_dmas(k // 8 + 1)
                                elif k == 24:
                                    emit_table_dmas(0)
                                elif k == 28:
                                    emit_wo_dmas()
                            elif b == 1 and c == 0 and k == 1:
                                emit_table_dmas(1)
                        vt_stage = vstg.tile([H, TCH], bf16, tag="vstage")
                        nc.scalar.activation(vt_stage, g[5], Copy)
                        # rope on DVE; for c=0 the diagonal needs kT(c)
                        # early, otherwise it is consumed last.
                        order = ([(0, 0), (4, -1), (1, 1), (2, 2), (3, 3)]
                                 if c == 0 else
                                 [(0, 0), (1, 1), (2, 2), (3, 3), (4, -1)])
                        for i, hq in order:
                            dst = kTs[c] if hq < 0 else qTs[c][:, hq, :]
                            rope_math(g[i], dst, cs, sn)
                        # previous chunk's outT normalize muls: emitted here
                        # so DVE never queues behind the ACT Ln/Exp chain.
                        emit_norm_muls(pending_muls)
                        pending_muls = []
                        # o-projection window: 24 of the previous chunk's 32
                        # tiles fill the PE while the release/rope chain
                        # drains on ACT/DVE; the last 8 are saved as in-
                        # attention filler for the per-head exp latencies.
                        fillers = (oproj_tile_emitters(pending)
                                   if pending is not None else [])
                        pending = None
                        for em in fillers[:24]:
                            em(po)
                        fillers = fillers[24:]
                    with ExitStack() as s2:
                        pt = s2.enter_context(
                            tc.tile_pool(name="pt", bufs=1, space="PSUM"))
                        fpo = s2.enter_context(
                            tc.tile_pool(name="fpo", bufs=1, space="PSUM"))
                        ps_s = s2.enter_context(
                            tc.tile_pool(name="ps_s", bufs=2, space="PSUM"))
                        ps_av = s2.enter_context(
                            tc.tile_pool(name="ps_av", bufs=2, space="PSUM"))
                        ps_l = s2.enter_context(
                            tc.tile_pool(name="ps_l", bufs=2, space="PSUM"))
                        fi = 0

                        def fill(n=1):
                            nonlocal fi
                            for _ in range(n):
                                if fi < len(fillers):
                                    fillers[fi](fpo)
                                    fi += 1

                        for j in range(NSUB):
                            tp = pt.tile([H, H], bf16, tag="vtp")
                            nc.tensor.transpose(
                                tp, vt_stage[:, j * H:(j + 1) * H], identity)
                            nc.scalar.activation(vs[c][:, j, :], tp, Copy)
                        n_st = (c + 1) * NSUB
                        outT_sb = otpool.tile([H, NHC, TCH], bf16, tag="outT")
                        av_all = avlpool.tile([H, NHC, TCH], bf16, tag="ava")
                        l_all = avlpool.tile([H, NHC, TCH], bf16, tag="la")
                        for h in range(NHC):
                            av_ps = ps_av.tile([H, TCH], f32, tag="av")
                            l_ps = ps_l.tile([H, TCH], f32, tag="lbc")

                            def scores_block(st):
                                j = st - c * NSUB
                                off = H * j if j > 0 else 0
                                kt = kTs[st // NSUB][
                                    :, (st % NSUB) * H:(st % NSUB + 1) * H]
                                sps = ps_s.tile([H, TCH], f32, tag="s")
                                nc.tensor.matmul(
                                    sps[:, off:], kt, qTs[c][:, h, off:],
                                    start=True, stop=True)
                                pT2 = ppool.tile([H, TCH], bf16, tag="p2")
                                if j >= 0:
                                    pT = pdiag.tile([H, TCH], bf16, tag="pd")
                                    nc.scalar.activation(
                                        pT[:, off:], sps[:, off:], Exp,
                                        scale=C_SM)
                                    nc.vector.tensor_mul(
                                        pT2[:, off:], pT[:, off:],
                                        masks[j][:, off:])
                                else:
                                    nc.scalar.activation(
                                        pT2, sps, Exp, scale=C_SM)
                                return pT2

                            def av_block(st, pT2):
                                j = st - c * NSUB
                                off = H * j if j > 0 else 0
                                nc.tensor.matmul(
                                    av_ps[:, off:],
                                    vs[st // NSUB][:, st % NSUB, :],
                                    pT2[:, off:],
                                    start=(st == 0), stop=(st == n_st - 1),
                                    skip_group_check=True)
                                nc.tensor.matmul(
                                    l_ps[:, off:], ones_sq, pT2[:, off:],
                                    start=(st == 0), stop=(st == n_st - 1),
                                    skip_group_check=True)

                            prev = scores_block(0)
                            for st in range(1, n_st):
                                cur = scores_block(st)
                                if st == 1 or st == c * NSUB:
                                    fill(1)
                                av_block(st - 1, prev)
                                prev = cur
                            av_block(n_st - 1, prev)
                            nc.vector.tensor_copy(av_all[:, h, :], av_ps)
                            nc.vector.tensor_copy(l_all[:, h, :], l_ps)
                        fill(len(fillers) - fi)
                        if b == B - 1 and c == NTCH - 1:
                            muls = normalize(av_all, l_all, outT_sb, (0, 4))
                            emit_norm_muls(muls)
                        else:
                            pending_muls = normalize(av_all, l_all, outT_sb,
                                                     (0, 4))
                        pending = (b, c, outT_sb)
            with ExitStack() as s3:
                po3 = s3.enter_context(
                    tc.tile_pool(name="po3", bufs=2, space="PSUM"))
                for em in oproj_tile_emitters(pending):
                    em(po3)

    nc.compile()
    return nc


_NC_CACHE = None


def kernel(x, wq, wk, wv, wo, positions):
    global _NC_CACHE
    import ml_dtypes
    from concourse.bass_utils import run_bass_kernel_spmd

    bf16 = ml_dtypes.bfloat16
    x = np.asarray(x, dtype=np.float32)
    wq = np.asarray(wq, dtype=np.float32)
    wk = np.asarray(wk, dtype=np.float32)
    wv = np.asarray(wv, dtype=np.float32)
    wo = np.asarray(wo, dtype=np.float32)
    positions = np.asarray(positions)

    xT = np.ascontiguousarray(x.reshape(TOK, D).T).astype(bf16)
    # rope tables, [H, B*T]: duplicated across partition halves; sin's
    # first half negated (see kernel docstring).
    fraction = 2.0 * np.arange(HH, dtype=np.float32) / H
    timescale = (THETA ** fraction).astype(np.float32)
    pos = positions.reshape(TOK).astype(np.float32)
    sinusoid = pos[None, :] / timescale[:, None]
    cos_h = np.cos(sinusoid).astype(np.float32)
    sin_h = np.sin(sinusoid).astype(np.float32)
    cosT = np.ascontiguousarray(np.concatenate([cos_h, cos_h], axis=0))
    sinST = np.ascontiguousarray(np.concatenate([-sin_h, sin_h], axis=0))

    if _NC_CACHE is None:
        _NC_CACHE = _build_bass()
    nc = _NC_CACHE

    in_maps = []
    for c in range(NCORES):
        in_maps.append({
            "xT": xT,
            "wq": np.ascontiguousarray(wq[c * NHC:(c + 1) * NHC]).astype(bf16),
            "wk": np.ascontiguousarray(wk[c]).astype(bf16),
            "wv": np.ascontiguousarray(wv[c]).astype(bf16),
            "wo": np.ascontiguousarray(wo[c * NHC:(c + 1) * NHC]).astype(bf16),
            "cosT": cosT,
            "sinST": sinST,
        })

    trace = os.environ.get("BASS_KERNEL_TRACE", "0") == "1"
    res = run_bass_kernel_spmd(nc, in_maps, list(range(NCORES)), trace=trace)
    global LAST_RESULTS
    LAST_RESULTS = res
    out = np.zeros((TOK, D), dtype=np.float32)
    for c in range(NCORES):
        out += res.results[c]["o_part"].astype(np.float32)
    return out.reshape(B, T, D)


LAST_RESULTS = None
